# revision 5
# baseline (speedup 1.0000x reference)
"""Trainium2 Bass kernel for a Bahdanau-attention GRU decoder (v3).

Reference computation (T=512, B=128, I=H=512, O=12, L=max_labels=16):
    s0 = tanh(x[0] @ ws);  out0 = s0 @ fc_w + fc_b
    U  = einsum('tbi,ih->tbh', x, ua)            # precomputed once
    per step:
        e  = einsum('tbh,h->tb', tanh(s @ wa + U), va)
        a  = softmax(e, axis=t)
        c  = einsum('tb,tbi->bi', a, x)
        r  = sigmoid(out @ wr + s @ ur + c @ cr)
        z  = sigmoid(out @ wz + s @ uz + c @ cz)
        sh = tanh(out @ w0 + (r*s) @ u0 + c @ c0)
        s  = (1-z)*s + z*sh;  out = s @ fc_w + fc_b
    returns [B, L, O]

Sharding: data-parallel over batch B across 8 cores (BL=16 per core), weights
replicated, no collectives.

Schedule: the 16 per-core batches split into 2 independent groups of 8.
ACT is the bottleneck (8 tanh slabs of [128, 8*512] per step = 28.8us), so
each group's serial softmax/gate/state tail is emitted INTERLEAVED between
the other group's slab instructions: the in-order ACT queue then runs
tail ops in the gaps between slab tanhs and the recurrence's serial chain
(state -> sWa -> V-add -> tanh) completes during the other group's last slab.

Layouts:
  x_nat [t%128, b, t//128, i] fp16 - context matmul lhsT chunks
  U_sb  [h%128, h//128, t, b] fp16 - b innermost so the V add is one
        2x-mode broadcast TensorTensor per (group, hc) slab
  e     [t%128, t//128, b] via V-chunks-as-lhsT matmuls with the va column
        as rhs (free=1: ~zero PE engine time, and e lands in exactly the
        layout the context needs for p, so softmax has no transposes)
  states TRANSPOSED only: sT [h%128, h//128, b], outT [O-pad, b];
        every gate matmul is weight-chunk lhsT x state-column rhs (free=8)
        producing [h, b] tiles, so the loop has no state transposes at all.
  softmax without max-subtraction: |e| <= ||va||_1 ~ 26 so exp stays within
        fp32/bf16 range; p-tilde = exp(e) goes STRAIGHT to bf16 (bf16 has
        fp32's exponent range) so the context runs on unnormalized weights
        and 1/S folds into the cT PSUM->SBUF copy - the whole sum/reciprocal
        chain computes off the critical path. x / xT / ua / ws are bf16 to
        match the context and U matmul operand dtypes.
"""

import numpy as np
from contextlib import ExitStack

import concourse.bass as bass
import concourse.mybir as mybir
import concourse.tile as tile
from concourse import bacc
from concourse.bass_utils import run_bass_kernel_spmd
from concourse.masks import make_identity

F32 = mybir.dt.float32
F16 = mybir.dt.float16
BF16 = mybir.dt.bfloat16
AF = mybir.ActivationFunctionType
ALU = mybir.AluOpType
AX = mybir.AxisListType

T, B, I, H, O = 512, 128, 512, 512, 12
P = 128
NCORES = 8
BL = B // NCORES        # 16 batches per core
HC = H // P             # 4 h-chunks
IC = I // P             # 4 i-chunks
TC = T // P             # 4 t-chunks
NG = 2                  # batch groups per core
GB = BL // NG           # 8 batches per group

WNAMES = ["w0", "wz", "wr", "ws", "wa", "ua", "va", "u0", "uz", "ur",
          "c0", "cz", "cr", "fc_w", "fc_b"]


def _build_decoder(ctx: ExitStack, tc_: tile.TileContext, L: int, io: dict):
    nc = tc_.nc
    x, out = io["x"], io["out"]

    const = ctx.enter_context(tc_.tile_pool(name="const", bufs=1))
    big = ctx.enter_context(tc_.tile_pool(name="big", bufs=1))
    state = ctx.enter_context(tc_.tile_pool(name="state", bufs=1))

    ident16 = const.tile([P, P], BF16)
    make_identity(nc, ident16[:])

    x_nat = big.tile([P, BL, TC, I], BF16)   # x[t%128, b, t//128, i]
    U_sb = big.tile([P, HC, T, BL], F16)     # U[h%128, h//128, t, b]

    sT32 = [state.tile([P, HC, GB], F32, name=f"sT32_{g}") for g in range(NG)]
    sT16 = [state.tile([P, HC, GB], F16, name=f"sT16_{g}") for g in range(NG)]
    outT16 = [state.tile([P, GB], F16, name=f"outT_{g}") for g in range(NG)]

    def load_pkh(pool, wname, kc, cast_dve, dtype=F16):
        """DRAM [K, H] fp32 -> SBUF [P, kc, H], staged in halves."""
        w16 = pool.tile([P, kc, H], dtype, name=f"{wname}_sb")
        src_ap = io[wname].rearrange("(c p) h -> p c h", p=P)
        half = kc // 2 if kc > 1 else kc
        for c0 in range(0, kc, half):
            tmp = wcast.tile([P, half, H], F32, tag="wload",
                             name=f"{wname}_f32_{c0}", bufs=2)
            nc.sync.dma_start(tmp[:], src_ap[:, c0:c0 + half, :])
            if cast_dve:
                nc.vector.tensor_copy(w16[:, c0:c0 + half, :], tmp[:])
            else:
                nc.scalar.copy(w16[:, c0:c0 + half, :], tmp[:])
        return w16

    # ---------------- setup ----------------
    with tc_.tile_pool(name="setup", bufs=2) as stp, \
         tc_.tile_pool(name="setup1", bufs=1) as stp1, \
         tc_.tile_pool(name="wcast", bufs=2) as wcast, \
         tc_.tile_pool(name="stpsA", bufs=2, space="PSUM") as stpsA, \
         tc_.tile_pool(name="stpsB", bufs=2, space="PSUM") as stpsB, \
         tc_.tile_pool(name="stpsC", bufs=1, space="PSUM") as stpsC:

        # DMA queue order matters (FIFO): everything the s0/out0/sWa chain
        # needs goes before the 16 big x transfers
        ua_sb = load_pkh(stp1, "ua", IC, cast_dve=True, dtype=BF16)

        # va / vasel selector (va in column b of each [P, GB] block)
        va_f32 = const.tile([P, HC], F32)
        nc.sync.dma_start(va_f32[:],
                          io["va"][:, 0].rearrange("(c p) -> p c", p=P))
        va16 = const.tile([P, HC], F16)
        nc.vector.tensor_copy(va16[:], va_f32[:])
        # softmax-sum helpers (all fp32; free<=8 matmuls so cost ~0):
        #   ones128: S4[(tc,b)] = sum_p p32[p, tc, b]
        #   bsel32:  S[b] = sum_tc S4[(tc, b)]
        #   ident8/ones8: replicate 1/S across partitions via diag matmul
        onesPP = const.tile([P, P], BF16)
        nc.vector.memset(onesPP[:], 1.0)
        for g in range(NG):
            nc.vector.memset(outT16[g][:], 0.0)

        # ---- per-batch: DMA x, cast fp16, transpose, U = (x @ ua)^T ----
        def load_batch(b):
            xdma = stp.tile([P, TC, I], F32, tag="xdma", name="xdma")
            nc.sync.dma_start(
                xdma[:], x[:, b, :].rearrange("(c p) i -> p c i", p=P))
            if b % 2 == 0:
                nc.vector.tensor_copy(x_nat[:, b, :, :], xdma[:])
            else:
                nc.scalar.copy(x_nat[:, b, :, :], xdma[:])
            xT_b = stp.tile([P, IC, T], BF16, tag="xT", name="xT_b")
            for ic in range(IC):
                tps = stpsA.tile([P, T], BF16, tag="xtp", name="xtp")
                for t_ in range(TC):
                    nc.tensor.transpose(
                        tps[:, t_ * P:(t_ + 1) * P],
                        x_nat[:, b, t_, ic * P:(ic + 1) * P], ident16[:])
                if ic % 2 == 0:
                    nc.vector.tensor_copy(xT_b[:, ic, :], tps[:])
                else:
                    nc.scalar.copy(xT_b[:, ic, :], tps[:])
            for hc in range(HC):
                ups = stpsB.tile([P, T], F32, tag="ups", name="ups")
                for ic in range(IC):
                    nc.tensor.matmul(
                        ups[:], ua_sb[:, ic, hc * P:(hc + 1) * P],
                        xT_b[:, ic, :],
                        start=(ic == 0), stop=(ic == IC - 1))
                if hc % 2 == 0:
                    nc.vector.tensor_copy(U_sb[:, hc, :, b], ups[:])
                else:
                    nc.scalar.copy(U_sb[:, hc, :, b], ups[:])

        for b in range(BL):
            load_batch(b)

        # ---- remaining weights (casts split DVE/ACT) ----
        wa_sb = load_pkh(const, "wa", HC, cast_dve=True, dtype=F32)
        ur_sb = load_pkh(const, "ur", HC, cast_dve=False)
        uz_sb = load_pkh(const, "uz", HC, cast_dve=True)
        u0_sb = load_pkh(const, "u0", HC, cast_dve=False)
        cr_sb = load_pkh(const, "cr", IC, cast_dve=True)
        cz_sb = load_pkh(const, "cz", IC, cast_dve=False)
        c0_sb = load_pkh(const, "c0", IC, cast_dve=True)
        ws_sb = load_pkh(stp1, "ws", IC, cast_dve=False, dtype=BF16)

        wsmall = {}
        for nm in ("wr", "wz", "w0"):
            tmp = wcast.tile([O, H], F32, tag="wload", name=f"{nm}_f32",
                             bufs=2)
            nc.sync.dma_start(tmp[:], io[nm])
            w16 = const.tile([P, H], F16, name=f"{nm}_sb")
            nc.vector.memset(w16[:], 0.0)
            nc.vector.tensor_copy(w16[:O, :], tmp[:])
            wsmall[nm] = w16

        fcw_sb = const.tile([P, HC, O], F32)
        nc.sync.dma_start(fcw_sb[:],
                          io["fc_w"].rearrange("(c p) o -> p c o", p=P))
        fcbT = const.tile([O, 1], F32)
        nc.sync.dma_start(fcbT[:], io["fc_b"][:, None])

        # ---- s0 = tanh(x0 @ ws) (transposed) ----
        x0_f32 = stp1.tile([BL, I], F32)
        nc.sync.dma_start(x0_f32[:], x[0, :, :])
        x0_f16 = stp1.tile([BL, I], BF16)
        nc.vector.tensor_copy(x0_f16[:], x0_f32[:])
        x0T = stp1.tile([P, IC, BL], BF16)
        x0tp = stpsA.tile([P, IC, BL], BF16, tag="xtp", name="x0tp")
        for ic in range(IC):
            nc.tensor.transpose(x0tp[:, ic, :],
                                x0_f16[:, ic * P:(ic + 1) * P],
                                ident16[:BL, :BL])
        nc.vector.tensor_copy(x0T[:], x0tp[:])

        s0T_ps = stpsC.tile([P, HC, BL], F32, name="s0T_ps")
        for hc in range(HC):
            for ic in range(IC):
                nc.tensor.matmul(
                    s0T_ps[:, hc, :], ws_sb[:, ic, hc * P:(hc + 1) * P],
                    x0T[:, ic, :], start=(ic == 0), stop=(ic == IC - 1))
        for g in range(NG):
            gs = slice(g * GB, (g + 1) * GB)
            nc.scalar.activation(sT16[g][:], s0T_ps[:, :, gs], AF.Tanh)
            nc.scalar.activation(sT32[g][:], s0T_ps[:, :, gs], AF.Tanh)


    # ---------------- step-loop pools ----------------
    work = ctx.enter_context(tc_.tile_pool(name="work", bufs=2))
    f16s = ctx.enter_context(tc_.tile_pool(name="f16s", bufs=2))
    vpool = ctx.enter_context(tc_.tile_pool(name="vpool", bufs=2))
    psE = ctx.enter_context(tc_.tile_pool(name="psE", bufs=1, space="PSUM"))
    psS = ctx.enter_context(tc_.tile_pool(name="psS", bufs=1, space="PSUM"))
    psN = ctx.enter_context(tc_.tile_pool(name="psN", bufs=1, space="PSUM"))

    # out0 via flipped fc (per group)
    for g in range(NG):
        otp = psS.tile([O, GB], F32, tag=f"B{g}", name=f"out0_ps{g}")
        for kc in range(HC):
            nc.tensor.matmul(otp[:], fcw_sb[:, kc, :], sT32[g][:, kc, :],
                             start=(kc == 0), stop=(kc == HC - 1))
        osb = work.tile([O, GB], F32, tag=f"osb{g}", name=f"out0_sb{g}")
        nc.vector.tensor_tensor(osb[:], otp[:],
                                fcbT[:, 0:1].to_broadcast((O, GB)), ALU.add)
        nc.sync.dma_start(
            out[0, g * GB:(g + 1) * GB, :].rearrange("b o -> o b"), osb[:])
        nc.vector.tensor_copy(outT16[g][:O, :], osb[:])

    swaT = [None] * NG   # fp16 sWa, broadcast operand of the V adds
    e_ps = [None] * NG

    swps = [None] * NG

    def emit_swa(g):
        """sWaT[h, b] = (s @ wa)^T for group g from the fp32 state (fp32
        lhsT matmuls cost 4 cyc/row but free=8, so ~nothing)."""
        swps[g] = psS.tile([P, HC, GB], F32, tag=f"A{g}", name=f"sw_ps{g}")
        for hc in range(HC):
            for kc in range(HC):
                nc.tensor.matmul(swps[g][:, hc, :],
                                 wa_sb[:, kc, hc * P:(hc + 1) * P],
                                 sT32[g][:, kc, :],
                                 start=(kc == 0), stop=(kc == HC - 1))
        swaT[g] = f16s.tile([P, HC, GB], F16, tag=f"swaT{g}",
                            name=f"swaT{g}")
        nc.vector.tensor_copy(swaT[g][:], swps[g][:])

    def emit_slabs(g, pieces=None):
        """V = tanh(U + sWa) and e-dot for group g; `pieces` are the other
        group's tail closures, interleaved one per hc slab."""
        gs = slice(g * GB, (g + 1) * GB)
        # e in [t%128, tc, b] layout (the layout p needs for the context):
        # lhsT = V t-chunks (data as weights), rhs = va column, free=1
        e_ps[g] = psE.tile([P, TC, GB], F32, tag=f"e{g}", name=f"e_ps{g}")
        for hc in range(HC):
            v = vpool.tile([P, T, GB], F16, tag=f"v{g}", name=f"v{g}_{hc}")
            bounds = (0, T)
            for h in range(len(bounds) - 1):
                hsl = slice(bounds[h], bounds[h + 1])
                TH = bounds[h + 1] - bounds[h]
                nc.vector.tensor_tensor(
                    v[:, hsl, :], U_sb[:, hc, hsl, gs],
                    swaT[g][:, hc, None, :].to_broadcast((P, TH, GB)),
                    ALU.add)
                nc.scalar.activation(v[:, hsl, :], v[:, hsl, :], AF.Tanh)
            # start only on the bank's first write (start pends-zero the
            # whole 2KB region; later first-writes land fresh, repeats
            # accumulate), stop only on the bank's last write
            for bi in range(GB):
                for t_ in range(TC):
                    nc.tensor.matmul(
                        e_ps[g][:, t_, bi:bi + 1],
                        v[:, t_ * P:(t_ + 1) * P, bi],
                        va16[:, hc:hc + 1],
                        start=(hc == 0 and bi == 0 and t_ == 0),
                        stop=(hc == HC - 1 and bi == GB - 1
                              and t_ == TC - 1))
            if pieces is not None:
                pieces[hc]()

    def make_tail(g, k):
        """Tail of group g, step k, split into 4 pieces. Writes sT/outT and
        (for k+1 < L) the next step's sWa."""
        gs = slice(g * GB, (g + 1) * GB)
        ts = {}

        def t1():  # softmax (no max: |e| <= ||va||_1, fp32/bf16-safe)
            # p-tilde = exp(e) straight to bf16 (bf16 has fp32's exponent
            # range, so no pre-normalization): the context runs on the
            # unnormalized weights and 1/S folds into the cT copy, taking
            # the whole S chain off the attention critical path
            pT = f16s.tile([P, TC, GB], BF16, tag=f"pT{g}", name=f"pT{g}")
            nc.scalar.activation(pT[:], e_ps[g][:], AF.Exp)
            S_ps = psN.tile([P, GB], F32, tag=f"N{g}", name=f"S_{g}")
            for t_ in range(TC):
                nc.tensor.matmul(S_ps[:], onesPP[:], pT[:, t_, :],
                                 start=(t_ == 0), stop=(t_ == TC - 1))
            Sinv = work.tile([P, GB], F32, tag=f"Si{g}", name=f"Si{g}")
            nc.vector.reciprocal(Sinv[:], S_ps[:])
            ts["pT"], ts["Sinv"] = pT, Sinv

        def t2():  # context (x_nat as lhsT, free=1) + r/z preacts + thrz
            pT = ts["pT"]
            cT_ps = psS.tile([P, IC, GB], F32, tag=f"A{g}", name=f"cT_ps{g}")
            for bi in range(GB):
                b = g * GB + bi
                for ic in range(IC):
                    for t_ in range(TC):
                        nc.tensor.matmul(
                            cT_ps[:, ic, bi:bi + 1],
                            x_nat[:, b, t_, ic * P:(ic + 1) * P],
                            pT[:, t_, bi:bi + 1],
                            start=(t_ == 0), stop=(t_ == TC - 1))
            cT = f16s.tile([P, IC, GB], F16, tag=f"cT{g}", name=f"cT{g}")
            nc.vector.tensor_tensor(
                cT[:], cT_ps[:],
                ts["Sinv"][:, None, :].to_broadcast((P, IC, GB)), ALU.mult)
            ts["cT"] = cT

            preRZ = psS.tile([P, 2, HC, GB], F32, tag=f"B{g}",
                             name=f"preRZ{g}")
            for j, (wn, usb, csb) in enumerate(
                    (("wr", ur_sb, cr_sb), ("wz", uz_sb, cz_sb))):
                for hc in range(HC):
                    hs = slice(hc * P, (hc + 1) * P)
                    nc.tensor.matmul(preRZ[:, j, hc, :], wsmall[wn][:, hs],
                                     outT16[g][:], start=True, stop=False)
                    for kc in range(HC):
                        nc.tensor.matmul(preRZ[:, j, hc, :],
                                         usb[:, kc, hs], sT16[g][:, kc, :],
                                         start=False, stop=False)
                    for ic in range(IC):
                        nc.tensor.matmul(preRZ[:, j, hc, :],
                                         csb[:, ic, hs], cT[:, ic, :],
                                         start=False, stop=(ic == IC - 1))
            # sigmoid(x) = 0.5*tanh(x/2) + 0.5; one tanh covers r and z
            thrz = work.tile([P, 2, HC, GB], F32, tag=f"th{g}",
                             name=f"thrz{g}")
            nc.scalar.activation(thrz[:], preRZ[:], AF.Tanh, scale=0.5)
            ts["thrz"] = thrz

        def t3():  # rs, z pre-products, h preact, sh tanh
            thrz, cT = ts["thrz"], ts["cT"]
            tmp = work.tile([P, HC, GB], F32, tag=f"rs32{g}", name=f"rs32{g}")
            nc.vector.scalar_tensor_tensor(
                out=tmp[:], in0=thrz[:, 0], scalar=0.5, in1=sT32[g][:],
                op0=ALU.mult, op1=ALU.mult)
            rsT = f16s.tile([P, HC, GB], F16, tag=f"rsT{g}", name=f"rsT{g}")
            nc.vector.scalar_tensor_tensor(
                out=rsT[:], in0=sT32[g][:], scalar=0.5, in1=tmp[:],
                op0=ALU.mult, op1=ALU.add)
            # z = 0.5*th_z + 0.5; zs = z*s; smzs = s - zs  (ready before sh)
            z32 = work.tile([P, HC, GB], F32, tag=f"z32{g}", name=f"z32{g}")
            nc.vector.tensor_scalar(z32[:], thrz[:, 1], 0.5, 0.5,
                                    ALU.mult, ALU.add)
            zs = work.tile([P, HC, GB], F32, tag=f"zs{g}", name=f"zs{g}")
            nc.vector.tensor_tensor(zs[:], z32[:], sT32[g][:], ALU.mult)
            smzs = work.tile([P, HC, GB], F32, tag=f"sm{g}", name=f"smzs{g}")
            nc.vector.tensor_tensor(smzs[:], sT32[g][:], zs[:], ALU.subtract)
            ts["z32"], ts["smzs"] = z32, smzs

            # sWa is linear: accumulate wa^T(s - z*s) now; wa^T(z*sh) lands
            # in t4 right after sh, shortening the recurrence boundary

            preH = psS.tile([P, HC, GB], F32, tag=f"B{g}", name=f"preH{g}")
            for hc in range(HC):
                hs = slice(hc * P, (hc + 1) * P)
                nc.tensor.matmul(preH[:, hc, :], wsmall["w0"][:, hs],
                                 outT16[g][:], start=True, stop=False)
                for ic in range(IC):
                    nc.tensor.matmul(preH[:, hc, :], c0_sb[:, ic, hs],
                                     cT[:, ic, :], start=False, stop=False)
                for kc in range(HC):
                    nc.tensor.matmul(preH[:, hc, :], u0_sb[:, kc, hs],
                                     rsT[:, kc, :],
                                     start=False, stop=(kc == HC - 1))
            shT = work.tile([P, HC, GB], F32, tag=f"sh{g}", name=f"shT{g}")
            nc.scalar.activation(shT[:], preH[:], AF.Tanh)
            ts["shT"] = shT

        def t4():  # finish next sWa, then state update, fc/out
            z32, smzs, shT = ts["z32"], ts["smzs"], ts["shT"]
            q = work.tile([P, HC, GB], F32, tag=f"q{g}", name=f"q{g}")
            nc.vector.tensor_tensor(q[:], z32[:], shT[:], ALU.mult)
            nc.vector.tensor_tensor(sT32[g][:], smzs[:], q[:], ALU.add)
            nc.vector.tensor_copy(sT16[g][:], sT32[g][:])
            if k < L - 1:
                emit_swa(g)

            otp = psS.tile([O, GB], F32, tag=f"B{g}", name=f"out_ps{g}")
            for kc in range(HC):
                nc.tensor.matmul(otp[:], fcw_sb[:, kc, :],
                                 sT32[g][:, kc, :],
                                 start=(kc == 0), stop=(kc == HC - 1))
            osb = work.tile([O, GB], F32, tag=f"osb{g}", name=f"out_sb{g}")
            nc.vector.tensor_tensor(
                osb[:], otp[:], fcbT[:, 0:1].to_broadcast((O, GB)), ALU.add)
            nc.sync.dma_start(
                out[k, gs, :].rearrange("b o -> o b"), osb[:])
            if k < L - 1:
                nc.vector.tensor_copy(outT16[g][:O, :], osb[:])

        return [t1, t2, t3, t4]

    # ---------------- interleaved decode schedule ----------------
    # half-step stream: slabs(B, k) carries tail(A, k); slabs(A, k+1)
    # carries tail(B, k); the recurrence chain of each group completes
    # during the other group's last slab.
    if L > 1:
        emit_swa(0)
        emit_swa(1)
        emit_slabs(0)
        for k in range(1, L):
            emit_slabs(1, pieces=make_tail(0, k))
            if k + 1 < L:
                emit_slabs(0, pieces=make_tail(1, k))
            else:
                for piece in make_tail(1, k):
                    piece()


_BUILT = {}


def _get_nc(L: int):
    if L in _BUILT:
        return _BUILT[L]
    nc = bacc.Bacc("TRN2", target_bir_lowering=False, debug=False,
                   enable_asserts=False, num_devices=NCORES)
    io = {}
    io["x"] = nc.dram_tensor("x", [T, BL, I], F32, kind="ExternalInput").ap()
    shapes = {"w0": [O, H], "wz": [O, H], "wr": [O, H], "ws": [I, H],
              "wa": [H, H], "ua": [I, H], "va": [H, 1], "u0": [H, H],
              "uz": [H, H], "ur": [H, H], "c0": [I, H], "cz": [I, H],
              "cr": [I, H], "fc_w": [H, O], "fc_b": [O]}
    for nm, shp in shapes.items():
        io[nm] = nc.dram_tensor(nm, shp, F32, kind="ExternalInput").ap()
    io["out"] = nc.dram_tensor("out", [L, BL, O], F32,
                               kind="ExternalOutput").ap()
    with tile.TileContext(nc) as tc_:
        with ExitStack() as ctx:
            _build_decoder(ctx, tc_, L, io)
    nc.compile()
    _BUILT[L] = (nc, io)
    return _BUILT[L]


def kernel(**inputs) -> np.ndarray:
    L = int(np.asarray(inputs["max_labels"]))
    nc, _ = _get_nc(L)
    x = np.ascontiguousarray(np.asarray(inputs["x"], dtype=np.float32))
    base = {nm: np.ascontiguousarray(np.asarray(inputs[nm], dtype=np.float32))
            for nm in WNAMES}
    base["fc_b"] = base["fc_b"].reshape(O)
    in_maps = []
    for c in range(NCORES):
        m = dict(base)
        m["x"] = np.ascontiguousarray(x[:, c * BL:(c + 1) * BL, :])
        in_maps.append(m)
    res = run_bass_kernel_spmd(nc, in_maps, core_ids=list(range(NCORES)))
    outs = [r["out"] for r in res.results]            # each [L, BL, O]
    full = np.concatenate([o.transpose(1, 0, 2) for o in outs], axis=0)
    return np.ascontiguousarray(full.astype(np.float32))


if __name__ == "__main__":
    import reference
    ins = reference.setup_inputs()
    got = kernel(**{k: np.asarray(v) if not isinstance(v, int) else v
                    for k, v in ins.items()})
    print("kernel output", got.shape, got.dtype)
